# revision 69
# baseline (speedup 1.0000x reference)
"""Sparse (block-diagonal) attention kernel for Trainium2, 8-core SPMD.

Reference computation (per query i in group g):
    qz = q @ Wq + bq                      (N, 256)
    kz = k @ Wk + bk                      (n, 128, 256)
    s[i, l] = <kz[g, l], qz[i]> / 16
    p = softmax(mask(s))
    out[i]  = sum_l p[i, l] * v[g, l]

Algebraic transform (exact under softmax shift invariance):
    <k@Wk + bk, qz> = <k, Wk @ qz> + <bk, qz>
The <bk, qz> term is constant per query row and drops out of the softmax,
so we compute u = ((q@Wq + bq)/16) @ Wk^T once (tiny) and score raw k
against u directly.

Layout/precision strategy (all host-side prep; the device streams k and v
exactly once):
  - k is pre-transposed on the host to kt[dh, d, g, l] (d on partitions)
    and cast to fp16 -> the score matmul needs no on-device transpose and
    runs at 1 cycle/column.
  - v is pre-transposed to v[l, g, d] fp16 (contraction dim l on
    partitions for the p @ v stage; V_FP8=1 switches it to fp8-e3m4 for
    ~3.4us more speed at rel-err 1.8e-2 - off by default, too close to
    the 2e-2 gate).
  - q^T, Wq, Wk^T are fp16; biases, softmax and all accumulations are f32.
  - All of k/v fits in SBUF, so every DMA is issued up-front across the
    three DMA-capable queues (SP, Pool, Act) and compute drains behind.
  - Groups and their query slices are sharded across the 8 cores; the
    small projection weights are replicated.
"""

import os
from contextlib import ExitStack

import numpy as np

N_CORES = 8
N_GROUPS = 1024
L = 128              # keys per group
R = 4                # queries per group
D = 256              # d_q = d_k = d_z = d_v
G_CORE = N_GROUPS // N_CORES      # 128 groups per core
Q_CORE = G_CORE * R               # 512 queries per core
GB = 16                           # groups per compute block
NBLK = G_CORE // GB               # 8 blocks per core
QB = GB * R                       # 64 query columns per block
SCALE = 1.0 / 16.0                # 1/sqrt(d_z)
NEG = -1.0e30                     # additive mask value

_CACHE = {}


def _build_bass():
    import concourse.tile as tile
    from concourse import bacc, mybir

    f32 = mybir.dt.float32
    fp16 = mybir.dt.float16
    u8 = mybir.dt.uint8
    vdt = mybir.dt.float8e3 if os.environ.get("V_FP8", "0") == "1" else fp16

    nc = bacc.Bacc(None, target_bir_lowering=False, debug=False)
    qt = nc.dram_tensor("qt", (D, Q_CORE), fp16, kind="ExternalInput")
    kt = nc.dram_tensor("kt", (2, 128, G_CORE, L), fp16, kind="ExternalInput")
    v = nc.dram_tensor("v", (L, G_CORE, D), vdt, kind="ExternalInput")
    mt = nc.dram_tensor("mt", (L, G_CORE), u8, kind="ExternalInput")
    w2 = nc.dram_tensor("w2", (2, 128, 2, D), fp16, kind="ExternalInput")  # Wq|Wk^T
    bq2 = nc.dram_tensor("bq2", (128, 2), f32, kind="ExternalInput")
    out = nc.dram_tensor("out", (Q_CORE, D), f32, kind="ExternalOutput")

    with tile.TileContext(nc) as tc, ExitStack() as ctx:
        singles = ctx.enter_context(tc.tile_pool(name="singles", bufs=1))
        ktp = ctx.enter_context(tc.tile_pool(name="ktp", bufs=3))
        vp = ctx.enter_context(tc.tile_pool(name="vp", bufs=NBLK))

        # ---- all DMAs, emitted so the first ones span all three queues.
        # kt is chunked per-queue (one big DMA each); v stays per-block so
        # the output stage unblocks as early as possible.  Keeping the
        # total DMA count low matters: DMA-completion signaling multiplexes
        # over 8 shared DMAHW semaphores in scheduled order, and collisions
        # chain DMAs across queues.
        kt_chunks = {}  # queue -> (tile, g0, ng)
        for g0, ng, eng in ((32, 48, nc.sync), (80, 48, nc.gpsimd)):
            t = ktp.tile([128, 2, ng, L], fp16, tag="kt")  # [d, dh, g, l]
            eng.dma_start(
                t, kt[:, :, g0 : g0 + ng, :].rearrange("h d g l -> d h g l")
            )
            kt_chunks[g0] = (t, ng)

        qt_sb = singles.tile([128, 2, Q_CORE], fp16)   # [c_in, c_half, i]
        nc.scalar.dma_start(qt_sb, qt[:].rearrange("(h p) i -> p h i", p=128))
        w2_sb = singles.tile([128, 2, 2, D], fp16)     # [p, half, which, z]
        nc.scalar.dma_start(w2_sb, w2[:].rearrange("h p w z -> p h w z"))
        wq_sb = w2_sb[:, :, 0, :]
        wkt_sb = w2_sb[:, :, 1, :]
        bq_sb = singles.tile([128, 2], f32)            # [z_in, z_half]
        nc.scalar.dma_start(bq_sb, bq2[:])
        mt_sb = singles.tile([128, G_CORE], u8)        # [l, g]
        nc.scalar.dma_start(mt_sb, mt[:])
        t = ktp.tile([128, 2, 32, L], fp16, tag="kt")
        nc.scalar.dma_start(t, kt[:, :, 0:32, :].rearrange("h d g l -> d h g l"))
        kt_chunks[0] = (t, 32)

        def _kt_view(b):
            for g0, (t, ng) in kt_chunks.items():
                if g0 <= b * GB < g0 + ng:
                    return t, b * GB - g0
            raise AssertionError

        # v per block: Act {0,1}, SP {2,3,7}, Pool {4,5,6}.  Act's second
        # v (block 1) is emitted AFTER the exp activations so the exps run
        # at data-ready time (~14us) instead of behind Act's queue (16.7) —
        # the whole output tail is a latency pipeline hanging off exp0.
        v_tiles = []
        vengs = (nc.scalar, nc.scalar, nc.sync, nc.sync,
                 nc.gpsimd, nc.gpsimd, nc.gpsimd, nc.sync)
        for b in range(NBLK):
            g0 = b * GB
            v_sb = vp.tile([128, GB, D], vdt, tag="v")        # [l, g, d]
            if b != 1:
                vengs[b].dma_start(v_sb, v[:, g0 : g0 + GB, :])
            v_tiles.append(v_sb)

        ones = singles.tile([128, 1], fp16)
        nc.vector.memset(ones, 1.0)
        # identity for PE transposes, built on GpSimd (keeps DMA queues free)
        ident_sb = singles.tile([128, 128], f32)
        nc.gpsimd.memset(ident_sb, 1.0)
        nc.gpsimd.affine_select(
            ident_sb,
            ident_sb,
            pattern=[[1, 128]],
            compare_op=mybir.AluOpType.is_equal,
            fill=0.0,
            base=0,
            channel_multiplier=-1,
        )
        # additive mask, transposed: (m - 1) * 1e30 -> 0 valid, -1e30 masked
        m_negT = singles.tile([128, G_CORE], f32)
        nc.vector.tensor_scalar(
            m_negT,
            mt_sb,
            scalar1=1.0,
            scalar2=-NEG,
            op0=mybir.AluOpType.subtract,
            op1=mybir.AluOpType.mult,
        )

        qzT = singles.tile([128, 2, Q_CORE], fp16)     # [z_in, z_half, i]
        ut = singles.tile([128, 2, Q_CORE], fp16)      # [d_in, d_half, i]

        # ---- preamble: u = ((q@Wq + bq)/16) @ Wk^T, stored transposed
        with tc.tile_pool(name="pre_ps", bufs=2, space="PSUM") as pre_ps:
            for zh in range(2):
                ps = pre_ps.tile([128, Q_CORE], f32, tag="pre")
                for ch in range(2):
                    nc.tensor.matmul(
                        ps,
                        lhsT=wq_sb[:, ch, zh * 128 : (zh + 1) * 128],
                        rhs=qt_sb[:, ch, :],
                        start=(ch == 0),
                        stop=(ch == 1),
                    )
                # qzT = (ps + bq) * 1/16, rounded to fp16
                nc.vector.tensor_scalar(
                    qzT[:, zh, :],
                    ps,
                    scalar1=bq_sb[:, zh : zh + 1],
                    scalar2=SCALE,
                    op0=mybir.AluOpType.add,
                    op1=mybir.AluOpType.mult,
                )
            for dh in range(2):
                ps = pre_ps.tile([128, Q_CORE], f32, tag="pre")
                for zh in range(2):
                    nc.tensor.matmul(
                        ps,
                        lhsT=wkt_sb[:, zh, dh * 128 : (dh + 1) * 128],
                        rhs=qzT[:, zh, :],
                        start=(zh == 0),
                        stop=(zh == 1),
                    )
                nc.vector.tensor_copy(ut[:, dh, :], ps)

        # ---- main compute pools ---------------------------------------
        # One PSUM bank-tile per block: st @ cols 0:64, sums @ 64:65,
        # ot @ 128:256, o_t @ 256:512.
        work = ctx.enter_context(tc.tile_pool(name="work", bufs=4))
        trw = ctx.enter_context(tc.tile_pool(name="trw", bufs=4))
        outp = ctx.enter_context(tc.tile_pool(name="outp", bufs=4))
        blk_ps = ctx.enter_context(tc.tile_pool(name="blk_ps", bufs=4, space="PSUM"))

        # Pair-granular tail: blocks (2p, 2p+1) share PSUM tiles and the
        # tail ops run once per 128 queries, halving cross-engine hops.
        NPAIR = NBLK // 2
        pairA, pairB, pm2_t, sm2_t = [], [], [], []

        # phase 1: scores + mask + exp, emitted for all blocks first so no
        # engine stream blocks a later block's scores behind a tail op.
        for p in range(NPAIR):
            A = blk_ps.tile([128, 512], f32, tag="blkA")
            B = blk_ps.tile([128, 256], f32, tag="blkB")
            s_m2 = work.tile([128, 2 * QB], f32, tag="s_m2")
            pm2 = work.tile([128, 2 * QB], fp16, tag="pm2")
            pairA.append(A)
            pairB.append(B)
            sm2_t.append(s_m2)
            pm2_t.append(pm2)
            for j in range(2):
                b = 2 * p + j
                q0 = b * QB
                kt_sb, goff = _kt_view(b)
                st = A[:, j * QB : (j + 1) * QB]
                for gi in range(GB):
                    for dh in range(2):
                        nc.tensor.matmul(
                            st[:, gi * R : (gi + 1) * R],
                            lhsT=kt_sb[:, dh, goff + gi, :],
                            rhs=ut[:, dh, q0 + gi * R : q0 + (gi + 1) * R],
                            start=(dh == 0),
                            stop=(dh == 1),
                        )
            g0 = 2 * p * GB
            nc.vector.tensor_tensor(
                s_m2.rearrange("p (g j) -> p g j", g=2 * GB),
                A[:, 0 : 2 * QB].rearrange("p (g j) -> p g j", g=2 * GB),
                m_negT[:, g0 : g0 + 2 * GB, None].to_broadcast((128, 2 * GB, R)),
                mybir.AluOpType.add,
            )
            nc.scalar.activation(pm2, s_m2, mybir.ActivationFunctionType.Exp)

        # block 1's v load, deliberately behind the exps in Act's stream
        nc.scalar.dma_start(v_tiles[1], v[:, GB : 2 * GB, :])

        # phase 2: per-pair softmax-normalize + output repack, emitted in
        # v-arrival order (p0's v1 lands late; in-order engines would
        # otherwise stall later pairs' tail ops behind it)
        for p in (1, 2, 0, 3):
            A, B, pm2 = pairA[p], pairB[p], pm2_t[p]
            sums = A[:, 2 * QB : 2 * QB + 1]
            nc.tensor.matmul(sums, lhsT=pm2, rhs=ones, start=True, stop=True)
            rrec = work.tile([128, 1], f32, tag="rrec")
            nc.vector.reciprocal(rrec, sums)

            # OT[dv, q] = v^T @ pm  (per group; contract l on partitions)
            ot = B[:, 0:256].rearrange("p (h q) -> p h q", h=2)  # [dv, dvh, q]
            for j in range(2):
                b = 2 * p + j
                v_sb = v_tiles[b]
                for gi in range(GB):
                    for dh in range(2):
                        qc = j * QB + gi * R
                        nc.tensor.matmul(
                            ot[:, dh, qc : qc + R],
                            lhsT=v_sb[:, gi, dh * 128 : (dh + 1) * 128],
                            rhs=pm2[:, qc : qc + R],
                            start=True,
                            stop=True,
                        )

            # repack OT -> out rows (q, dv), normalized by 1/sums.
            # p0's copy/scale ride Act (free after the exps; GpSimd has no
            # PSUM port) so they don't serialize with the final pair's on
            # DVE.
            ot_sb = trw.tile([128, 2, 128], f32, tag="ot_sb")
            if p == 0:
                nc.scalar.copy(ot_sb, ot)
            else:
                nc.vector.tensor_copy(ot_sb, ot)
            o_t = A[:, 256:512].rearrange("p (h d) -> p h d", h=2)  # [q, dvh, dv]
            for dh in range(2):
                nc.tensor.transpose(o_t[:, dh, :], ot_sb[:, dh, :], ident_sb)
            out_sb = outp.tile([128, D], f32, tag="out_sb")
            if p == 0:
                nc.scalar.activation(
                    out_sb.rearrange("p (h d) -> p h d", h=2),
                    o_t,
                    mybir.ActivationFunctionType.Identity,
                    scale=rrec,
                )
            else:
                nc.vector.tensor_scalar_mul(
                    out_sb.rearrange("p (h d) -> p h d", h=2), o_t, rrec
                )
            oeng = (nc.scalar, nc.scalar, nc.sync, nc.gpsimd)[p]
            oeng.dma_start(out[p * 128 : (p + 1) * 128, :], out_sb)

    nc.compile()
    return nc


def _get_nc():
    if "nc" not in _CACHE:
        _CACHE["nc"] = _build_bass()
    return _CACHE["nc"]


def _make_in_maps(inputs):
    fp16 = np.float16
    if os.environ.get("V_FP8", "0") == "1":
        import ml_dtypes

        v_np_dt = ml_dtypes.float8_e3m4
    else:
        v_np_dt = fp16

    q = np.asarray(inputs["q"], dtype=np.float32)
    k = np.asarray(inputs["k"], dtype=np.float32)
    v = np.asarray(inputs["v"], dtype=np.float32)
    m = np.asarray(inputs["m"]).astype(np.uint8)
    # w2[h, p, w, z]: w=0 -> Wq[h*128+p, z], w=1 -> Wk^T[h*128+p, z]
    w2 = np.ascontiguousarray(
        np.stack(
            [
                np.asarray(inputs["Wq"], dtype=np.float32).reshape(2, 128, D),
                np.asarray(inputs["Wk"], dtype=np.float32).T.reshape(2, 128, D),
            ],
            axis=2,
        )
    ).astype(fp16)
    bq = np.asarray(inputs["bq"], dtype=np.float32)
    bq2 = np.ascontiguousarray(bq.reshape(2, 128).T)

    in_maps = []
    for c in range(N_CORES):
        gs, ge = c * G_CORE, (c + 1) * G_CORE
        qs, qe = c * Q_CORE, (c + 1) * Q_CORE
        qt_c = np.ascontiguousarray(q[qs:qe].T).astype(fp16)  # (D, Q_CORE)
        # kt[dh, d, g, l]: d-major transpose of k
        kt_c = np.ascontiguousarray(
            k[gs:ge].transpose(2, 0, 1).reshape(2, 128, G_CORE, L)
        ).astype(fp16)
        v_c = np.ascontiguousarray(v[gs:ge].transpose(1, 0, 2)).astype(v_np_dt)  # (L,G,D)
        mt_c = np.ascontiguousarray(m[gs:ge].T)  # (L, G)
        in_maps.append(
            {
                "qt": qt_c,
                "kt": kt_c,
                "v": v_c,
                "mt": mt_c,
                "w2": w2,
                "bq2": bq2,
            }
        )
    return in_maps


def run(inputs, trace=False):
    """Run the SPMD kernel; returns (full_output, exec_time_ns_or_None)."""
    from concourse.bass_utils import run_bass_kernel_spmd

    nc = _get_nc()
    in_maps = _make_in_maps(inputs)
    res = run_bass_kernel_spmd(
        nc, in_maps, core_ids=list(range(N_CORES)), trace=trace
    )
    outs = [res.results[c]["out"] for c in range(N_CORES)]
    full = np.concatenate(outs, axis=0).astype(np.float32)
    return full, res.exec_time_ns


def kernel(**inputs) -> np.ndarray:
    full, _ = run(inputs, trace=False)
    return full


# revision 73
# speedup vs baseline: 1.0143x; 1.0143x over previous
"""Sparse (block-diagonal) attention kernel for Trainium2, 8-core SPMD.

Reference computation (per query i in group g):
    qz = q @ Wq + bq                      (N, 256)
    kz = k @ Wk + bk                      (n, 128, 256)
    s[i, l] = <kz[g, l], qz[i]> / 16
    p = softmax(mask(s))
    out[i]  = sum_l p[i, l] * v[g, l]

Algebraic transform (exact under softmax shift invariance):
    <k@Wk + bk, qz> = <k, Wk @ qz> + <bk, qz>
The <bk, qz> term is constant per query row and drops out of the softmax,
so we compute u = ((q@Wq + bq)/16) @ Wk^T once (tiny) and score raw k
against u directly.

Layout/precision strategy (all host-side prep; the device streams k and v
exactly once):
  - k is pre-transposed on the host to kt[dh, d, g, l] (d on partitions)
    and cast to fp16 -> the score matmul needs no on-device transpose and
    runs at 1 cycle/column.
  - v is pre-transposed to v[l, g, d] fp16 (contraction dim l on
    partitions for the p @ v stage; V_FP8=1 switches it to fp8-e3m4 for
    ~3.4us more speed at rel-err 1.8e-2 - off by default, too close to
    the 2e-2 gate).
  - q^T, Wq, Wk^T are fp16; biases, softmax and all accumulations are f32.
  - All of k/v fits in SBUF, so every DMA is issued up-front across the
    three DMA-capable queues (SP, Pool, Act) and compute drains behind.
  - Groups and their query slices are sharded across the 8 cores; the
    small projection weights are replicated.
"""

import os
from contextlib import ExitStack

import numpy as np

N_CORES = 8
N_GROUPS = 1024
L = 128              # keys per group
R = 4                # queries per group
D = 256              # d_q = d_k = d_z = d_v
G_CORE = N_GROUPS // N_CORES      # 128 groups per core
Q_CORE = G_CORE * R               # 512 queries per core
GB = 16                           # groups per compute block
NBLK = G_CORE // GB               # 8 blocks per core
QB = GB * R                       # 64 query columns per block
SCALE = 1.0 / 16.0                # 1/sqrt(d_z)
NEG = -1.0e30                     # additive mask value

_CACHE = {}


def _build_bass():
    import concourse.tile as tile
    from concourse import bacc, mybir

    f32 = mybir.dt.float32
    fp16 = mybir.dt.float16
    u8 = mybir.dt.uint8
    vdt = mybir.dt.float8e3 if os.environ.get("V_FP8", "0") == "1" else fp16

    nc = bacc.Bacc(None, target_bir_lowering=False, debug=False)
    qt = nc.dram_tensor("qt", (D, Q_CORE), fp16, kind="ExternalInput")
    kt = nc.dram_tensor("kt", (2, 128, G_CORE, L), fp16, kind="ExternalInput")
    v = nc.dram_tensor("v", (L, G_CORE, D), vdt, kind="ExternalInput")
    mt = nc.dram_tensor("mt", (L, G_CORE), u8, kind="ExternalInput")
    w2 = nc.dram_tensor("w2", (2, 128, 2, D), fp16, kind="ExternalInput")  # Wq|Wk^T
    bq2 = nc.dram_tensor("bq2", (128, 2), f32, kind="ExternalInput")
    out = nc.dram_tensor("out", (Q_CORE, D), f32, kind="ExternalOutput")

    with tile.TileContext(nc) as tc, ExitStack() as ctx:
        singles = ctx.enter_context(tc.tile_pool(name="singles", bufs=1))
        ktp = ctx.enter_context(tc.tile_pool(name="ktp", bufs=3))
        vp = ctx.enter_context(tc.tile_pool(name="vp", bufs=NBLK))

        # ---- all DMAs, emitted so the first ones span all three queues.
        # kt is chunked per-queue (one big DMA each); v stays per-block so
        # the output stage unblocks as early as possible.  Keeping the
        # total DMA count low matters: DMA-completion signaling multiplexes
        # over 8 shared DMAHW semaphores in scheduled order, and collisions
        # chain DMAs across queues.
        kt_chunks = {}  # queue -> (tile, g0, ng)
        for g0, ng, eng in ((32, 48, nc.sync), (80, 48, nc.gpsimd)):
            t = ktp.tile([128, 2, ng, L], fp16, tag="kt")  # [d, dh, g, l]
            eng.dma_start(
                t, kt[:, :, g0 : g0 + ng, :].rearrange("h d g l -> d h g l")
            )
            kt_chunks[g0] = (t, ng)

        qt_sb = singles.tile([128, 2, Q_CORE], fp16)   # [c_in, c_half, i]
        nc.scalar.dma_start(qt_sb, qt[:].rearrange("(h p) i -> p h i", p=128))
        w2_sb = singles.tile([128, 2, 2, D], fp16)     # [p, half, which, z]
        nc.scalar.dma_start(w2_sb, w2[:].rearrange("h p w z -> p h w z"))
        wq_sb = w2_sb[:, :, 0, :]
        wkt_sb = w2_sb[:, :, 1, :]
        bq_sb = singles.tile([128, 2], f32)            # [z_in, z_half]
        nc.scalar.dma_start(bq_sb, bq2[:])
        mt_sb = singles.tile([128, G_CORE], u8)        # [l, g]
        nc.scalar.dma_start(mt_sb, mt[:])
        t = ktp.tile([128, 2, 32, L], fp16, tag="kt")
        nc.scalar.dma_start(t, kt[:, :, 0:32, :].rearrange("h d g l -> d h g l"))
        kt_chunks[0] = (t, 32)

        def _kt_view(b):
            for g0, (t, ng) in kt_chunks.items():
                if g0 <= b * GB < g0 + ng:
                    return t, b * GB - g0
            raise AssertionError

        # v per block: Act {0,1}, SP {2,3,7}, Pool {4,5,6}.  Act's second
        # v (block 1) is emitted AFTER the exp activations so the exps run
        # at data-ready time (~14us) instead of behind Act's queue (16.7) —
        # the whole output tail is a latency pipeline hanging off exp0.
        v_tiles = []
        vengs = (nc.scalar, nc.scalar, nc.sync, nc.sync,
                 nc.gpsimd, nc.gpsimd, nc.gpsimd, nc.sync)
        for b in range(NBLK):
            g0 = b * GB
            v_sb = vp.tile([128, GB, D], vdt, tag="v")        # [l, g, d]
            if b != 1:
                vengs[b].dma_start(v_sb, v[:, g0 : g0 + GB, :])
            v_tiles.append(v_sb)

        ones = singles.tile([128, 1], fp16)
        nc.vector.memset(ones, 1.0)
        # identity for PE transposes, built on GpSimd (keeps DMA queues free)
        ident_sb = singles.tile([128, 128], f32)
        nc.gpsimd.memset(ident_sb, 1.0)
        nc.gpsimd.affine_select(
            ident_sb,
            ident_sb,
            pattern=[[1, 128]],
            compare_op=mybir.AluOpType.is_equal,
            fill=0.0,
            base=0,
            channel_multiplier=-1,
        )
        # additive mask, transposed: (m - 1) * 1e30 -> 0 valid, -1e30 masked
        m_negT = singles.tile([128, G_CORE], f32)
        nc.vector.tensor_scalar(
            m_negT,
            mt_sb,
            scalar1=1.0,
            scalar2=-NEG,
            op0=mybir.AluOpType.subtract,
            op1=mybir.AluOpType.mult,
        )

        qzT = singles.tile([128, 2, Q_CORE], fp16)     # [z_in, z_half, i]
        ut = singles.tile([128, 2, Q_CORE], fp16)      # [d_in, d_half, i]

        # ---- preamble: u = ((q@Wq + bq)/16) @ Wk^T, stored transposed
        with tc.tile_pool(name="pre_ps", bufs=2, space="PSUM") as pre_ps:
            for zh in range(2):
                ps = pre_ps.tile([128, Q_CORE], f32, tag="pre")
                for ch in range(2):
                    nc.tensor.matmul(
                        ps,
                        lhsT=wq_sb[:, ch, zh * 128 : (zh + 1) * 128],
                        rhs=qt_sb[:, ch, :],
                        start=(ch == 0),
                        stop=(ch == 1),
                    )
                # qzT = (ps + bq) * 1/16, rounded to fp16
                nc.vector.tensor_scalar(
                    qzT[:, zh, :],
                    ps,
                    scalar1=bq_sb[:, zh : zh + 1],
                    scalar2=SCALE,
                    op0=mybir.AluOpType.add,
                    op1=mybir.AluOpType.mult,
                )
            for dh in range(2):
                ps = pre_ps.tile([128, Q_CORE], f32, tag="pre")
                for zh in range(2):
                    nc.tensor.matmul(
                        ps,
                        lhsT=wkt_sb[:, zh, dh * 128 : (dh + 1) * 128],
                        rhs=qzT[:, zh, :],
                        start=(zh == 0),
                        stop=(zh == 1),
                    )
                nc.vector.tensor_copy(ut[:, dh, :], ps)

        # ---- main compute pools ---------------------------------------
        # One PSUM bank-tile per block: st @ cols 0:64, sums @ 64:65,
        # ot @ 128:256, o_t @ 256:512.
        work = ctx.enter_context(tc.tile_pool(name="work", bufs=4))
        trw = ctx.enter_context(tc.tile_pool(name="trw", bufs=4))
        outp = ctx.enter_context(tc.tile_pool(name="outp", bufs=4))
        blk_ps = ctx.enter_context(tc.tile_pool(name="blk_ps", bufs=4, space="PSUM"))

        # Pair-granular tail: blocks (2p, 2p+1) share PSUM tiles and the
        # tail ops run once per 128 queries, halving cross-engine hops.
        NPAIR = NBLK // 2
        pairA, pairB, pm2_t, sm2_t = [], [], [], []

        # phase 1: scores + mask + exp, emitted for all blocks first so no
        # engine stream blocks a later block's scores behind a tail op.
        for p in range(NPAIR):
            A = blk_ps.tile([128, 512], f32, tag="blkA")
            B = blk_ps.tile([128, 256], f32, tag="blkB")
            s_m2 = work.tile([128, 2 * QB], f32, tag="s_m2")
            pm2 = work.tile([128, 2 * QB], fp16, tag="pm2")
            pairA.append(A)
            pairB.append(B)
            sm2_t.append(s_m2)
            pm2_t.append(pm2)
            for j in range(2):
                b = 2 * p + j
                q0 = b * QB
                kt_sb, goff = _kt_view(b)
                st = A[:, j * QB : (j + 1) * QB]
                for gi in range(GB):
                    for dh in range(2):
                        nc.tensor.matmul(
                            st[:, gi * R : (gi + 1) * R],
                            lhsT=kt_sb[:, dh, goff + gi, :],
                            rhs=ut[:, dh, q0 + gi * R : q0 + (gi + 1) * R],
                            start=(dh == 0),
                            stop=(dh == 1),
                        )
            g0 = 2 * p * GB
            nc.vector.tensor_tensor(
                s_m2.rearrange("p (g j) -> p g j", g=2 * GB),
                A[:, 0 : 2 * QB].rearrange("p (g j) -> p g j", g=2 * GB),
                m_negT[:, g0 : g0 + 2 * GB, None].to_broadcast((128, 2 * GB, R)),
                mybir.AluOpType.add,
            )
            nc.scalar.activation(pm2, s_m2, mybir.ActivationFunctionType.Exp)

        # block 1's v load, deliberately behind the exps in Act's stream
        nc.scalar.dma_start(v_tiles[1], v[:, GB : 2 * GB, :])

        # phase 2: per-pair softmax-normalize + output repack, emitted in
        # v-arrival order (p0's v1 lands late; in-order engines would
        # otherwise stall later pairs' tail ops behind it)
        for p in (1, 2, 0, 3):
            A, B, pm2 = pairA[p], pairB[p], pm2_t[p]
            sums = A[:, 2 * QB : 2 * QB + 1]
            nc.tensor.matmul(sums, lhsT=pm2, rhs=ones, start=True, stop=True)
            rrec = work.tile([128, 1], f32, tag="rrec")
            nc.vector.reciprocal(rrec, sums)

            # OT[dv, q] = v^T @ pm  (per group; contract l on partitions)
            ot = B[:, 0:256].rearrange("p (h q) -> p h q", h=2)  # [dv, dvh, q]
            for j in range(2):
                b = 2 * p + j
                v_sb = v_tiles[b]
                for gi in range(GB):
                    for dh in range(2):
                        qc = j * QB + gi * R
                        nc.tensor.matmul(
                            ot[:, dh, qc : qc + R],
                            lhsT=v_sb[:, gi, dh * 128 : (dh + 1) * 128],
                            rhs=pm2[:, qc : qc + R],
                            start=True,
                            stop=True,
                        )

            # repack OT -> out rows (q, dv), normalized by 1/sums.
            # p0's copy/scale ride Act (free after the exps; GpSimd has no
            # PSUM port) so they don't serialize with the final pair's on
            # DVE.
            ot_sb = trw.tile([128, 2, 128], f32, tag="ot_sb")
            if p == 0:
                nc.scalar.copy(ot_sb, ot)
            else:
                nc.vector.tensor_copy(ot_sb, ot)
            o_t = A[:, 256:512].rearrange("p (h d) -> p h d", h=2)  # [q, dvh, dv]
            for dh in range(2):
                nc.tensor.transpose(o_t[:, dh, :], ot_sb[:, dh, :], ident_sb)
            out_sb = outp.tile([128, D], f32, tag="out_sb")
            if p == 0:
                nc.scalar.activation(
                    out_sb.rearrange("p (h d) -> p h d", h=2),
                    o_t,
                    mybir.ActivationFunctionType.Identity,
                    scale=rrec,
                )
            else:
                nc.vector.tensor_scalar_mul(
                    out_sb.rearrange("p (h d) -> p h d", h=2), o_t, rrec
                )
            oeng = (nc.scalar, nc.scalar, nc.gpsimd, nc.scalar)[p]
            oeng.dma_start(out[p * 128 : (p + 1) * 128, :], out_sb)

    nc.compile()
    return nc


def _get_nc():
    if "nc" not in _CACHE:
        _CACHE["nc"] = _build_bass()
    return _CACHE["nc"]


def _make_in_maps(inputs):
    fp16 = np.float16
    if os.environ.get("V_FP8", "0") == "1":
        import ml_dtypes

        v_np_dt = ml_dtypes.float8_e3m4
    else:
        v_np_dt = fp16

    q = np.asarray(inputs["q"], dtype=np.float32)
    k = np.asarray(inputs["k"], dtype=np.float32)
    v = np.asarray(inputs["v"], dtype=np.float32)
    m = np.asarray(inputs["m"]).astype(np.uint8)
    # w2[h, p, w, z]: w=0 -> Wq[h*128+p, z], w=1 -> Wk^T[h*128+p, z]
    w2 = np.ascontiguousarray(
        np.stack(
            [
                np.asarray(inputs["Wq"], dtype=np.float32).reshape(2, 128, D),
                np.asarray(inputs["Wk"], dtype=np.float32).T.reshape(2, 128, D),
            ],
            axis=2,
        )
    ).astype(fp16)
    bq = np.asarray(inputs["bq"], dtype=np.float32)
    bq2 = np.ascontiguousarray(bq.reshape(2, 128).T)

    in_maps = []
    for c in range(N_CORES):
        gs, ge = c * G_CORE, (c + 1) * G_CORE
        qs, qe = c * Q_CORE, (c + 1) * Q_CORE
        qt_c = np.ascontiguousarray(q[qs:qe].T).astype(fp16)  # (D, Q_CORE)
        # kt[dh, d, g, l]: d-major transpose of k
        kt_c = np.ascontiguousarray(
            k[gs:ge].transpose(2, 0, 1).reshape(2, 128, G_CORE, L)
        ).astype(fp16)
        v_c = np.ascontiguousarray(v[gs:ge].transpose(1, 0, 2)).astype(v_np_dt)  # (L,G,D)
        mt_c = np.ascontiguousarray(m[gs:ge].T)  # (L, G)
        in_maps.append(
            {
                "qt": qt_c,
                "kt": kt_c,
                "v": v_c,
                "mt": mt_c,
                "w2": w2,
                "bq2": bq2,
            }
        )
    return in_maps


def run(inputs, trace=False):
    """Run the SPMD kernel; returns (full_output, exec_time_ns_or_None)."""
    from concourse.bass_utils import run_bass_kernel_spmd

    nc = _get_nc()
    in_maps = _make_in_maps(inputs)
    res = run_bass_kernel_spmd(
        nc, in_maps, core_ids=list(range(N_CORES)), trace=trace
    )
    outs = [res.results[c]["out"] for c in range(N_CORES)]
    full = np.concatenate(outs, axis=0).astype(np.float32)
    return full, res.exec_time_ns


def kernel(**inputs) -> np.ndarray:
    full, _ = run(inputs, trace=False)
    return full
